# revision 1
# baseline (speedup 1.0000x reference)
"""Trainium2 Bass kernel for nn_EquivariantDecoder (GNN message passing).

Sharding: nodes are split into 8 contiguous ranges of 6272 (= 49 tiles of
128); each core owns the edges whose dst lands in its range, so per-node
segment sums are core-local (no collectives). Edges are sorted by dst on
the host and padded so every (core, node-tile) group holds exactly K
tiles of 128 edge slots; the K is baked into the traced program.

Device work per core:
  edge path:  z = m_ij @ W1 via fp8(e4m3) DoubleRow matmuls (inputs
              pre-scaled x16 on host, rescaled by the activation's 1/256)
              s1 = silu(z + b1)                     (scalar engine, bf16)
              u  = s1_0*w2_0 + s1_1*w2_1            (DVE, folds W2)
              w  = colsum(u) via a ones-rhs matmul  (PE, 1 ld per tile)
              one-hot is host-precomputed (edge assignment is static) and
              streamed from HBM as fp8 (exact 0/1; padding rows all-zero)
              scatter-sum of (w+b2)*rel via one-hot matmul into PSUM
  node path:  alpha = silu(h @ vgW1 + vgb1) @ vgW2 + vgb2, batched 4
              node tiles per matmul/activation group
              out = sum_k alpha_k * vel_k + scatter_sum * (1/max(cnt,1))
"""

import sys

import numpy as np

try:
    import concourse.bass as bass  # noqa: F401
except Exception:  # pragma: no cover
    sys.path.insert(0, "/opt/trn_rl_repo")

import concourse.bass as bass
import concourse.mybir as mybir
from concourse.bass_utils import run_bass_kernel_spmd
from concourse.tile import TileContext
from concourse.vector_clock import ScopedClock

N_NODES = 50000
N_EDGES = 800000
H = 256
N_CORES = 8
NT = 49                 # node tiles per core
NPC = NT * 128          # 6272 nodes per core
N_PAD = N_CORES * NPC   # 50176
P = 128
NG = 8                  # node tiles batched per node-path group

BF16 = mybir.dt.bfloat16
BF16_NP = mybir.dt.np(BF16)
F8 = mybir.dt.float8e4
F8_NP = mybir.dt.np(F8)
F32 = mybir.dt.float32
AF = mybir.ActivationFunctionType
AF_USED = AF.Silu   # sim tests may override (interp lacks Silu)
OP = mybir.AluOpType
DR = mybir.MatmulPerfMode.DoubleRow
MM_SCALE = 16.0         # host pre-scale on m_ij and W1 (undone by 1/256)


# ---------------------------------------------------------------------------
# Walrus on this toolchain rejects >2 sync waits on the TileContext tail
# drain ("Too many sync wait commands"); split them across SP NOPs.
def _patched_drain_and_barrier(self, tick_clock, wait_clock):
    drain_inst = self.nc.sync.drain()
    wait_clock.add_sem_waits(
        drain_inst.ins, ScopedClock({None: tick_clock.global_clock})
    )
    si = drain_inst.ins.sync_info
    if si is not None and si.on_wait and len(si.on_wait) > 1:
        extra = list(si.on_wait[1:])
        del si.on_wait[1:]
        for w in extra:
            nop = self.nc.sync.nop(nofuse=True, hint="drain_wait_split")
            nsi = nop.ins.sync_info
            if nsi is None:
                nop.ins.sync_info = mybir.SyncInfo(on_wait=[w], on_update=[])
            else:
                nsi.on_wait.append(w)

    self.nc.all_engine_barrier()
    assert self.sems is not None
    popped = self.nc._tile_sem_poison_stack.pop()
    assert popped is self._sem_poison
    self.nc.clear_and_free_semaphores(list(self.sems.allocated().values()))
    self.nc.all_engine_barrier()


TileContext._drain_and_barrier = _patched_drain_and_barrier


def _split_excess_waits(nc, maxw: int = 1):
    """Walrus rejects >maxw sync waits on one instruction; move the excess
    onto NOPs inserted just before, on the same engine (same-queue program
    order makes this equivalent)."""
    n_split = 0
    for f in nc.m.functions:
        for b in f.blocks:
            out = []
            for inst in b.instructions:
                si = inst.sync_info
                if si is not None and si.on_wait and len(si.on_wait) > maxw:
                    extra = list(si.on_wait[: -maxw])
                    del si.on_wait[: -maxw]
                    for i in range(0, len(extra), maxw):
                        nop = mybir.InstNoOp(
                            name=f"{inst.name}-wsplit{i}",
                            engine=inst.engine,
                            sync_info=mybir.SyncInfo(
                                on_wait=extra[i:i + maxw], on_update=[]),
                            bass_nofuse=True,
                        )
                        out.append(nop)
                    n_split += 1
                out.append(inst)
            b.instructions[:] = out
    return n_split
# ---------------------------------------------------------------------------


def _build_program(K: int, b2: float):
    """Trace the single-core SPMD program for a fixed K (edge tiles per
    node-tile group)."""
    ET = NT * K                      # edge tiles per core
    n_mac = (ET + 7) // 8            # macros of up to 8 edge tiles
    n_sup = (n_mac + 1) // 2         # supertiles of 2 macros (1 DMA each)

    nc = bass.Bass()

    mijT = nc.dram_tensor("mijT", [n_sup, P, 4096], F8, kind="ExternalInput")
    ohT = nc.dram_tensor("ohT", [n_sup, P, 2048], F8, kind="ExternalInput")
    rel_d = nc.dram_tensor("rel", [P, ET * 3], F32, kind="ExternalInput")
    hT = nc.dram_tensor("hT", [NT, P, 2 * P], BF16, kind="ExternalInput")
    velg_d = nc.dram_tensor("velg", [P, NT * 16], F32, kind="ExternalInput")
    w1dr_d = nc.dram_tensor("w1dr", [2, P, 2 * P], F8, kind="ExternalInput")
    w2c_d = nc.dram_tensor("w2c", [2, P, 1], F32, kind="ExternalInput")
    b1t_d = nc.dram_tensor("b1t", [2, P, 1], F32, kind="ExternalInput")
    vgw1b_d = nc.dram_tensor("vgw1b", [2, 2, P, P], BF16, kind="ExternalInput")
    vgw2t_d = nc.dram_tensor("vgw2t", [2, P, 5], BF16, kind="ExternalInput")
    vgb1t_d = nc.dram_tensor("vgb1t", [2, P, 1], F32, kind="ExternalInput")
    onesb2_d = nc.dram_tensor("onesb2", [1, 133], BF16, kind="ExternalInput")
    onesp_d = nc.dram_tensor("onesp", [P, 1], BF16, kind="ExternalInput")
    out_d = nc.dram_tensor("out", [P, NT * 3], F32, kind="ExternalOutput")

    with TileContext(nc) as tc:
        with (
            tc.tile_pool(name="const", bufs=1) as cpool,
            tc.tile_pool(name="rhs", bufs=4) as rhs_pool,
            tc.tile_pool(name="s1", bufs=4) as s1_pool,
            tc.tile_pool(name="small", bufs=6) as sm_pool,
            tc.tile_pool(name="oh", bufs=6) as oh_pool,
            tc.tile_pool(name="nodes", bufs=3) as nd_pool,
            tc.tile_pool(name="ps_mm1", bufs=2, space="PSUM") as ps1_pool,
            tc.tile_pool(name="ps_w", bufs=2, space="PSUM") as psw_pool,
            tc.tile_pool(name="ps_sc", bufs=2, space="PSUM") as pssc_pool,
        ):
            # ---- constants (small ones first; mijT sup DMAs stream in-loop)
            w1 = [cpool.tile([P, 2 * P], F8, tag=f"w1_{hh}", name=f"w1_{hh}")
                  for hh in range(2)]
            vgw1 = [[cpool.tile([P, P], BF16, tag=f"vgw1_{kk}{hh}",
                                name=f"vgw1_{kk}{hh}")
                     for hh in range(2)] for kk in range(2)]
            for hh in range(2):
                nc.sync.dma_start(w1[hh][:], w1dr_d[hh, :, :])
            for kk in range(2):
                for hh in range(2):
                    nc.gpsimd.dma_start(vgw1[kk][hh][:], vgw1b_d[kk, hh, :, :])
            w2c = [cpool.tile([P, 1], F32, tag=f"w2c_{hh}", name=f"w2c_{hh}")
                   for hh in range(2)]
            b1 = [cpool.tile([P, 1], F32, tag=f"b1_{hh}", name=f"b1_{hh}")
                  for hh in range(2)]
            vgw2 = [cpool.tile([P, 5], BF16, tag=f"vgw2_{hh}", name=f"vgw2_{hh}")
                    for hh in range(2)]
            vgb1 = [cpool.tile([P, 1], F32, tag=f"vgb1_{hh}", name=f"vgb1_{hh}")
                    for hh in range(2)]
            for hh in range(2):
                nc.scalar.dma_start(w2c[hh][:], w2c_d[hh, :, :])
                nc.scalar.dma_start(b1[hh][:], b1t_d[hh, :, :])
                nc.gpsimd.dma_start(vgw2[hh][:], vgw2t_d[hh, :, :])
                nc.gpsimd.dma_start(vgb1[hh][:], vgb1t_d[hh, :, :])
            onesb2 = cpool.tile([1, 133], BF16, tag="onesb2")
            nc.gpsimd.dma_start(onesb2[:], onesb2_d[0, :][None, :])
            onesp = cpool.tile([P, 1], BF16, tag="onesp")
            nc.scalar.dma_start(onesp[:], onesp_d[:, :])

            rel = cpool.tile([P, ET * 3], F32, tag="rel")
            nc.scalar.dma_start(rel[:], rel_d[:, :])
            velg = cpool.tile([P, NT * 16], F32, tag="velg")
            nc.scalar.dma_start(velg[:], velg_d[:, :])

            # all node features resident in SBUF: one DMA, no per-group loads
            hTall = cpool.tile([P, NT * 2 * P], BF16, tag="hTall")
            nc.scalar.dma_start(
                hTall[:].rearrange("p (t c) -> p t c", t=NT),
                hT[:, :, :].rearrange("t p c -> p t c"))

            # packed output, one column triple per node tile; single DMA at end
            outbuf = cpool.tile([P, NT * 3], F32, tag="outbuf")

            relv = rel.rearrange("p (t f) -> p t f", f=3)
            rhv = hTall.rearrange("p (t kk n) -> p t kk n", kk=2, n=P)

            # group tile holding geom for up to NG node tiles
            geomg_ref = [None]

            def node_group(g0: int, T: int):
                """Finish node tiles g0..g0+T-1: vel-gate MLP + combine."""
                geomg = geomg_ref[0]
                psn = [ps1_pool.tile([P, NG * P], F32, tag="ps_mm1", name="psn")
                       for _ in range(2)]
                n_nch = (T * P + 511) // 512
                for hh in range(2):
                    for ch in range(n_nch):
                        tl, th = ch * 4, min(T, ch * 4 + 4)
                        for kk in range(2):
                            nc.tensor.matmul(
                                psn[hh][:, tl * P:th * P],
                                vgw1[kk][hh][:],
                                rhv[:, g0 + tl:g0 + th, kk, :],
                                start=(kk == 0), stop=(kk == 1))
                s1n = [nd_pool.tile([P, NG * P], BF16, tag=f"s1n_{hh}",
                                    name=f"s1n_{hh}")
                       for hh in range(2)]
                for hh in range(2):
                    nc.scalar.activation(s1n[hh][:, 0:T * P],
                                         psn[hh][:, 0:T * P], AF_USED,
                                         bias=vgb1[hh][:, 0:1], scale=1.0)
                for t in range(T):
                    nt = g0 + t
                    psa = psw_pool.tile([P, 8], F32, tag="ps_w", name="psa")
                    for hh in range(2):
                        nc.tensor.matmul(psa[:, 0:5],
                                         s1n[hh][:, t * P:(t + 1) * P],
                                         vgw2[hh][:],
                                         start=(hh == 0), stop=False)
                    nc.tensor.matmul(psa[:, 0:5], onesb2[:, 0:128],
                                     onesb2[:, 128:133], start=False, stop=True)

                    # out[:, j] = geom[:, j] + sum_k alpha[k] * vel[j, k]
                    scratch = sm_pool.tile([P, 15], F32, tag="scratch")
                    vbase = nt * 16
                    velg_v = velg[:, vbase:vbase + 15].rearrange(
                        "p (j k) -> p j k", k=5)
                    nc.vector.tensor_tensor(
                        scratch[:].rearrange("p (j k) -> p j k", k=5),
                        velg_v,
                        psa[:, None, 0:5].broadcast_to([P, 3, 5]),
                        op=OP.mult)
                    acc = sm_pool.tile([P, 3], F32, tag="acc")
                    nc.vector.tensor_reduce(
                        acc[:, :, None],
                        scratch[:].rearrange("p (j k) -> p j k", k=5),
                        axis=mybir.AxisListType.X, op=OP.add)
                    nc.vector.tensor_add(outbuf[:, nt * 3:nt * 3 + 3],
                                         acc[:], geomg[:, t * 3:t * 3 + 3])

            # ---- software-pipelined edge-path macro loop ------------------
            # PE program order per iteration m:
            #   mm1(m), u-mm(m-1), scatter(m-2)
            # so the PE never waits on the scalar/DVE chain of the current
            # macro (act -> u-build -> msg) and stays continuously busy --
            # the TRN2 PE only reaches its full 2.4 GHz p-state after 3us of
            # uninterrupted execution; any idle gap resets it to 1.2 GHz.
            w1v = [w1[hh].rearrange("p (kk m) -> p kk m", kk=2)
                   for hh in range(2)]
            sup = None
            ohs = None
            stA = {}   # m -> state for u-mm+msg stage
            stB = {}   # m -> state for scatter stage
            ps_sc = None

            def head(m):
                nonlocal sup, ohs
                t0 = m * 8
                G = min(8, ET - t0)          # real edge tiles in this macro
                W = G * P                    # macro width in edges
                si, sm = divmod(m, 2)
                if sm == 0:
                    sup = rhs_pool.tile([P, 4096], F8, tag="sup", name="sup")
                    nc.sync.dma_start(sup[:], mijT[si, :, :])
                    ohs = oh_pool.tile([P, 2048], F8, tag="oh", name="ohs")
                    nc.sync.dma_start(ohs[:], ohT[si, :, :])
                supv = sup[:, sm * 2048:(sm + 1) * 2048].rearrange(
                    "p (kk e) -> p kk e", kk=2)
                ps1 = [ps1_pool.tile([P, 1024], F32, tag="ps_mm1", name="ps1")
                       for _ in range(2)]
                n_ch = (W + 511) // 512
                for hh in range(2):
                    for ch in range(n_ch):
                        cw = min(512, W - ch * 512)
                        nc.tensor.matmul(
                            ps1[hh][:, ch * 512:ch * 512 + cw],
                            w1v[hh],
                            supv[:, :, ch * 512:ch * 512 + cw],
                            start=True, stop=True, perf_mode=DR)
                s1 = [s1_pool.tile([P, 1024], BF16, tag=f"s1_{hh}",
                                   name=f"s1_{hh}")
                      for hh in range(2)]
                for hh in range(2):
                    nc.scalar.activation(s1[hh][:, 0:W], ps1[hh][:, 0:W],
                                         AF_USED,
                                         bias=b1[hh][:, 0:1],
                                         scale=1.0 / (MM_SCALE * MM_SCALE))
                # u = s1_0*w2_0 + s1_1*w2_1  (DVE; folds the W2 contraction
                # halves so the PE reduction is a single ones-rhs matmul).
                # tensor_scalar runs 4x on bf16 SBUF; tensor_tensor 2x.
                u0 = s1_pool.tile([P, 1024], BF16, tag="u0", name="u0")
                nc.vector.tensor_scalar(u0[:, 0:W], s1[0][:, 0:W],
                                        w2c[0][:, 0:1], None, op0=OP.mult)
                t1 = s1_pool.tile([P, 1024], BF16, tag="t1", name="t1")
                nc.vector.tensor_scalar(t1[:, 0:W], s1[1][:, 0:W],
                                        w2c[1][:, 0:1], None, op0=OP.mult)
                u1 = s1_pool.tile([P, 1024], BF16, tag="u1", name="u1")
                nc.vector.tensor_add(u1[:, 0:W], t1[:, 0:W], u0[:, 0:W])
                stA[m] = (u1, G, t0)

            def stage_umm(m):
                u1, G, t0 = stA.pop(m)
                # per-tile w sums: psw[:, c] = sum_h u[h, tile c]
                psw = psw_pool.tile([P, 8], F32, tag="ps_w")
                for c in range(G):
                    nc.tensor.matmul(psw[:, c:c + 1],
                                     u1[:, c * P:(c + 1) * P],
                                     onesp[:], start=True, stop=True)
                # msg for all chunks in one op: [P, G, 3] = (w + b2) * rel
                msg = sm_pool.tile([P, 24], BF16, tag="msg")
                nc.vector.scalar_tensor_tensor(
                    msg[:, 0:3 * G].rearrange("p (c f) -> p c f", f=3),
                    psw[:, 0:G, None].broadcast_to([P, G, 3]),
                    float(b2),
                    relv[:, t0:t0 + G, 0:3],
                    op0=OP.add, op1=OP.mult)
                stB[m] = msg

            def stage_scatter(m, ohs_m):
                nonlocal ps_sc
                msg = stB.pop(m)
                t0 = m * 8
                G = min(8, ET - t0)
                sm = m % 2
                for c in range(G):
                    t = t0 + c
                    nt, j = divmod(t, K)
                    if j == 0:
                        ps_sc = pssc_pool.tile([P, 3], F32, tag="ps_sc")
                    ohc = ohs_m[:, sm * 1024 + c * P:sm * 1024 + (c + 1) * P]
                    nc.tensor.matmul(ps_sc[:], ohc,
                                     msg[:, 3 * c:3 * c + 3],
                                     start=(j == 0), stop=(j == K - 1))
                    if j == K - 1:
                        g0 = (nt // NG) * NG
                        if nt % NG == 0:
                            geomg_ref[0] = sm_pool.tile(
                                [P, NG * 3], F32, tag="geomg", name="geomg")
                        # geom = scatter_sum * inv_count (velg col 15)
                        nc.vector.tensor_scalar(
                            geomg_ref[0][:, (nt - g0) * 3:(nt - g0) * 3 + 3],
                            ps_sc[:, 0:3],
                            velg[:, nt * 16 + 15:nt * 16 + 16],
                            None, op0=OP.mult)
                        if nt % NG == NG - 1 or nt == NT - 1:
                            node_group(g0, nt - g0 + 1)

            oh_hist = {}
            for m in range(n_mac + 2):
                if m < n_mac:
                    head(m)
                    oh_hist[m] = ohs
                if m >= 1 and m - 1 < n_mac:
                    stage_umm(m - 1)
                if m >= 2:
                    stage_scatter(m - 2, oh_hist.pop(m - 2))

            nc.sync.dma_start(out_d[:, :], outbuf[:])

    _split_excess_waits(nc)
    return nc


def _preprocess(inputs: dict):
    """Shard + lay out all per-core device inputs. Returns (in_maps, K, b2)."""
    h = np.asarray(inputs["h"], np.float32)
    m_ij = np.asarray(inputs["m_ij"], np.float32)
    x = np.asarray(inputs["x"], np.float32)
    vel_all = np.asarray(inputs["vel_all"], np.float32)
    ei = np.asarray(inputs["edge_index"])
    src = ei[0].astype(np.int64)
    dst = ei[1].astype(np.int64)

    counts = np.bincount(dst, minlength=N_NODES).astype(np.float32)
    invc = (1.0 / np.maximum(counts, 1.0)).astype(np.float32)

    order = np.argsort(dst, kind="stable")
    dst_s = dst[order]
    src_s = src[order]
    g = dst_s // P                       # global 128-node group, 0..391
    n_groups = N_PAD // P                # 392
    cg = np.bincount(g, minlength=n_groups)
    K = max(1, int(-(-cg.max() // P)))   # ceil(max group)/128
    ET = NT * K

    gstart = np.zeros(n_groups, np.int64)
    gstart[1:] = np.cumsum(cg)[:-1]
    within = np.arange(N_EDGES, dtype=np.int64) - gstart[g]
    slot = g * (K * P) + within          # slot in global [392, K*128] layout

    Sg = n_groups * K * P
    colidx = np.full(Sg, -1.0, np.float32)
    colidx[slot] = (dst_s % P).astype(np.float32)
    relp = np.zeros((Sg, 3), np.float32)
    relp[slot] = x[src_s] - x[dst_s]
    mijp = np.zeros((Sg, H), F8_NP)
    mijp[slot] = (m_ij[order] * MM_SCALE).astype(F8_NP)

    # padded node tensors
    hp = np.zeros((N_PAD, H), np.float32)
    hp[:N_NODES] = h
    velp = np.zeros((N_PAD, 5, 3), np.float32)
    velp[:N_NODES] = vel_all
    invp = np.ones(N_PAD, np.float32)
    invp[:N_NODES] = invc

    # weights (shared by all cores)
    w1 = np.asarray(inputs["ew_W1"], np.float32)
    b1 = np.asarray(inputs["ew_b1"], np.float32)
    w2 = np.asarray(inputs["ew_W2"], np.float32)
    b2 = float(np.asarray(inputs["ew_b2"], np.float32)[0])
    vgw1 = np.asarray(inputs["vg_W1"], np.float32)
    vgb1 = np.asarray(inputs["vg_b1"], np.float32)
    vgw2 = np.asarray(inputs["vg_W2"], np.float32)
    vgb2 = np.asarray(inputs["vg_b2"], np.float32)

    # w1dr[hh][p, kk*128+m] = 16*W1[kk*128+p, hh*128+m]  (DoubleRow layout)
    w1s = (w1 * MM_SCALE).reshape(2, P, 2, P)            # [kk, p, hh, m]
    w1dr = np.ascontiguousarray(
        w1s.transpose(2, 1, 0, 3)).reshape(2, P, 2 * P).astype(F8_NP)
    w2c = w2.reshape(2, P, 1).astype(np.float32).copy()
    b1t = b1.reshape(2, P, 1).copy()
    vgw1b = vgw1.reshape(2, P, 2, P).transpose(0, 2, 1, 3).astype(BF16_NP).copy()
    vgw2t = vgw2.reshape(2, P, 5).astype(BF16_NP).copy()
    vgb1t = vgb1.reshape(2, P, 1).copy()
    onesb2 = np.zeros((1, 133), BF16_NP)
    onesb2[0, :P] = 1.0
    onesb2[0, P:P + 5] = vgb2.astype(BF16_NP)
    onesp = np.ones((P, 1), BF16_NP)

    mijp = mijp.reshape(N_CORES, ET, P, H)
    relp = relp.reshape(N_CORES, ET, P, 3)
    colidx = colidx.reshape(N_CORES, ET, P)

    n_mac = (ET + 7) // 8
    n_sup = (n_mac + 1) // 2
    in_maps = []
    for k in range(N_CORES):
        # mijT supertiles: [n_sup, 128, 4096]; free = (mac, kk, tile, e),
        # partition = h within kk half
        b = mijp[k].transpose(0, 2, 1).reshape(ET, 2, P, P)
        full = np.zeros((n_sup * 16, 2, P, P), F8_NP)
        full[:ET] = b
        mijT = np.ascontiguousarray(
            full.reshape(n_sup, 2, 8, 2, P, P).transpose(0, 4, 1, 3, 2, 5)
        ).reshape(n_sup, P, 4096)

        rel = np.ascontiguousarray(
            relp[k].transpose(1, 0, 2)).reshape(P, ET * 3)

        # host-built one-hot: ohT[sup][p, (mac, tile, f)] = (col[t, p] == f)
        oh = (colidx[k][:, :, None] ==
              np.arange(P, dtype=np.float32)[None, None, :])  # [ET, P, 128]
        ohfull = np.zeros((n_sup * 16, P, P), F8_NP)
        ohfull[:ET] = oh.astype(F8_NP)
        ohT = np.ascontiguousarray(
            ohfull.reshape(n_sup, 16, P, P).transpose(0, 2, 1, 3)
        ).reshape(n_sup, P, 2048)

        hk = hp[k * NPC:(k + 1) * NPC].reshape(NT, P, H)
        hTk = np.ascontiguousarray(
            hk.transpose(0, 2, 1).reshape(NT, 2, P, P).transpose(0, 2, 1, 3)
        ).reshape(NT, P, 2 * P).astype(BF16_NP)

        # velg cols per node tile: [comp j, gate k] at 5*j+k, inv_count at 15
        vg = np.empty((P, NT, 16), np.float32)
        vg[:, :, 0:15] = (velp[k * NPC:(k + 1) * NPC]
                          .reshape(NT, P, 5, 3).transpose(1, 0, 3, 2)
                          .reshape(P, NT, 15))
        vg[:, :, 15] = invp[k * NPC:(k + 1) * NPC].reshape(NT, P).T
        velg = np.ascontiguousarray(vg).reshape(P, NT * 16)

        in_maps.append({
            "mijT": mijT,
            "ohT": ohT,
            "rel": rel,
            "hT": hTk,
            "velg": velg,
            "w1dr": w1dr,
            "w2c": w2c,
            "b1t": b1t,
            "vgw1b": vgw1b,
            "vgw2t": vgw2t,
            "vgb1t": vgb1t,
            "onesb2": onesb2,
            "onesp": onesp,
        })
    return in_maps, K, b2


def unpack_out(arr: np.ndarray) -> np.ndarray:
    """[128, NT*3] packed per-core output -> [NPC, 3]."""
    return arr.reshape(P, NT, 3).transpose(1, 0, 2).reshape(NPC, 3)


def kernel(**inputs) -> np.ndarray:
    in_maps, K, b2 = _preprocess(inputs)
    nc = _build_program(K, b2)
    res = run_bass_kernel_spmd(nc, in_maps, list(range(N_CORES)))
    parts = [unpack_out(res.results[k]["out"]) for k in range(N_CORES)]
    return np.concatenate(parts, axis=0)[:N_NODES].astype(np.float32)



# revision 3
# speedup vs baseline: 1.0011x; 1.0011x over previous
"""Trainium2 Bass kernel for nn_EquivariantDecoder (GNN message passing).

Sharding: nodes are split into 8 contiguous ranges of 6272 (= 49 tiles of
128); each core owns the edges whose dst lands in its range, so per-node
segment sums are core-local (no collectives).

v2 layout: each core sorts its 49 node-groups by edge-tile count
descending; schedule slot j gets K_j = max over cores of the j-th largest
per-group tile count, so one SPMD program covers all cores with ~2% edge
padding (host un-permutes the per-slot geom output at the end).

Device work per core:
  edge path:  z = m_ij @ W1 via fp8(e4m3) DoubleRow matmuls (inputs
              pre-scaled x16 on host, rescaled by the activation's 1/256)
              s1 = silu(z + b1)                     (scalar engine, bf16)
              t1 = s1_1*w2_1 (DVE TS); u1 = s1_0*w2_0 + t1 (DVE STT)
              psw = colsum(u1) via ones-rhs matmuls (PE, 1 per tile)
              msg = (psw + b2) * rel                (DVE STT, bf16)
              scatter: flipped one-hot matmul: lhsT = msg [128e, 3],
              rhs = host-built one-hot [128e, 128n] fp8 -> psum [3, 128n]
              accumulated per slot; banks of 4 slots copied to SBUF.
  node path:  alpha = silu(h @ vgW1 + vgb1) @ vgW2 + vgb2, batched 8
              node tiles per group; out = sum_k alpha_k * vel_k
              (geom mean + add folded into the host finalize).
"""

import sys

import numpy as np

try:
    import concourse.bass as bass  # noqa: F401
except Exception:  # pragma: no cover
    sys.path.insert(0, "/opt/trn_rl_repo")

import concourse.bass as bass
import concourse.mybir as mybir
from concourse.bass_utils import run_bass_kernel_spmd
from concourse.tile import TileContext
from concourse.vector_clock import ScopedClock

N_NODES = 50000
N_EDGES = 800000
H = 256
N_CORES = 8
NT = 49                 # node tiles (=groups) per core
NPC = NT * 128          # 6272 nodes per core
N_PAD = N_CORES * NPC   # 50176
P = 128
NG = 8                  # node tiles batched per node-path group

BF16 = mybir.dt.bfloat16
BF16_NP = mybir.dt.np(BF16)
F8 = mybir.dt.float8e4
F8_NP = mybir.dt.np(F8)
F32 = mybir.dt.float32
AF = mybir.ActivationFunctionType
AF_USED = AF.Silu   # sim tests may override (interp lacks Silu)
OP = mybir.AluOpType
DR = mybir.MatmulPerfMode.DoubleRow
MM_SCALE = 16.0         # host pre-scale on m_ij and W1 (undone by 1/256)


# ---------------------------------------------------------------------------
# Walrus on this toolchain rejects >2 sync waits on the TileContext tail
# drain ("Too many sync wait commands"); split them across SP NOPs.
def _patched_drain_and_barrier(self, tick_clock, wait_clock):
    drain_inst = self.nc.sync.drain()
    wait_clock.add_sem_waits(
        drain_inst.ins, ScopedClock({None: tick_clock.global_clock})
    )
    si = drain_inst.ins.sync_info
    if si is not None and si.on_wait and len(si.on_wait) > 1:
        extra = list(si.on_wait[1:])
        del si.on_wait[1:]
        for w in extra:
            nop = self.nc.sync.nop(nofuse=True, hint="drain_wait_split")
            nsi = nop.ins.sync_info
            if nsi is None:
                nop.ins.sync_info = mybir.SyncInfo(on_wait=[w], on_update=[])
            else:
                nsi.on_wait.append(w)

    self.nc.all_engine_barrier()
    assert self.sems is not None
    popped = self.nc._tile_sem_poison_stack.pop()
    assert popped is self._sem_poison
    self.nc.clear_and_free_semaphores(list(self.sems.allocated().values()))
    self.nc.all_engine_barrier()


TileContext._drain_and_barrier = _patched_drain_and_barrier


def _split_excess_waits(nc, maxw: int = 1):
    """Walrus rejects >maxw sync waits on one instruction; move the excess
    onto NOPs inserted just before, on the same engine (same-queue program
    order makes this equivalent)."""
    n_split = 0
    for f in nc.m.functions:
        for b in f.blocks:
            out = []
            for inst in b.instructions:
                si = inst.sync_info
                if si is not None and si.on_wait and len(si.on_wait) > maxw:
                    extra = list(si.on_wait[: -maxw])
                    del si.on_wait[: -maxw]
                    for i in range(0, len(extra), maxw):
                        nop = mybir.InstNoOp(
                            name=f"{inst.name}-wsplit{i}",
                            engine=inst.engine,
                            sync_info=mybir.SyncInfo(
                                on_wait=extra[i:i + maxw], on_update=[]),
                            bass_nofuse=True,
                        )
                        out.append(nop)
                    n_split += 1
                out.append(inst)
            b.instructions[:] = out
    return n_split
# ---------------------------------------------------------------------------


def _build_program(Ks: list[int], b2: float):
    """Trace the single-core SPMD program. Ks[j] = edge tiles in schedule
    slot j (shared across cores)."""
    ET = sum(Ks)                     # edge tiles per core (multiple of 8)
    n_mac = ET // 8                  # macros of 8 edge tiles
    n_sup = (n_mac + 1) // 2         # supertiles of 2 macros (1 DMA each)

    # tile -> (slot, idx-in-slot, slot-size)
    slot_of = []
    for j, K in enumerate(Ks):
        for i in range(K):
            slot_of.append((j, i, K))

    nc = bass.Bass()

    mijT = nc.dram_tensor("mijT", [n_sup, P, 4096], F8, kind="ExternalInput")
    ohT = nc.dram_tensor("ohT", [n_sup, P, 2048], F8, kind="ExternalInput")
    rel_d = nc.dram_tensor("rel", [P, ET * 3], F32, kind="ExternalInput")
    hT = nc.dram_tensor("hT", [NT, P, 2 * P], BF16, kind="ExternalInput")
    velg_d = nc.dram_tensor("velg", [P, NT * 15], F32, kind="ExternalInput")
    w1dr_d = nc.dram_tensor("w1dr", [2, P, 2 * P], F8, kind="ExternalInput")
    w2c_d = nc.dram_tensor("w2c", [2, P, 1], F32, kind="ExternalInput")
    b1t_d = nc.dram_tensor("b1t", [2, P, 1], F32, kind="ExternalInput")
    vgw1b_d = nc.dram_tensor("vgw1b", [2, 2, P, P], BF16, kind="ExternalInput")
    vgw2t_d = nc.dram_tensor("vgw2t", [2, P, 5], BF16, kind="ExternalInput")
    vgb1t_d = nc.dram_tensor("vgb1t", [2, P, 1], F32, kind="ExternalInput")
    onesb2_d = nc.dram_tensor("onesb2", [1, 133], BF16, kind="ExternalInput")
    onesp_d = nc.dram_tensor("onesp", [P, 1], BF16, kind="ExternalInput")
    out_d = nc.dram_tensor("out", [P, NT * 3], F32, kind="ExternalOutput")
    geo_d = nc.dram_tensor("geo", [3, NT * P], F32, kind="ExternalOutput")

    with TileContext(nc) as tc:
        with (
            tc.tile_pool(name="const", bufs=1) as cpool,
            tc.tile_pool(name="rhs", bufs=4) as rhs_pool,
            tc.tile_pool(name="s1", bufs=4) as s1_pool,
            tc.tile_pool(name="small", bufs=6) as sm_pool,
            tc.tile_pool(name="oh", bufs=6) as oh_pool,
            tc.tile_pool(name="nodes", bufs=3) as nd_pool,
            tc.tile_pool(name="ps_mm1", bufs=2, space="PSUM") as ps1_pool,
            tc.tile_pool(name="ps_w", bufs=2, space="PSUM") as psw_pool,
            tc.tile_pool(name="ps_geo", bufs=2, space="PSUM") as psg_pool,
        ):
            # ---- constants (small ones first; mijT sup DMAs stream in-loop)
            w1 = [cpool.tile([P, 2 * P], F8, tag=f"w1_{hh}", name=f"w1_{hh}")
                  for hh in range(2)]
            vgw1 = [[cpool.tile([P, P], BF16, tag=f"vgw1_{kk}{hh}",
                                name=f"vgw1_{kk}{hh}")
                     for hh in range(2)] for kk in range(2)]
            for hh in range(2):
                nc.sync.dma_start(w1[hh][:], w1dr_d[hh, :, :])
            for kk in range(2):
                for hh in range(2):
                    nc.gpsimd.dma_start(vgw1[kk][hh][:], vgw1b_d[kk, hh, :, :])
            w2c = [cpool.tile([P, 1], F32, tag=f"w2c_{hh}", name=f"w2c_{hh}")
                   for hh in range(2)]
            b1 = [cpool.tile([P, 1], F32, tag=f"b1_{hh}", name=f"b1_{hh}")
                  for hh in range(2)]
            vgw2 = [cpool.tile([P, 5], BF16, tag=f"vgw2_{hh}", name=f"vgw2_{hh}")
                    for hh in range(2)]
            vgb1 = [cpool.tile([P, 1], F32, tag=f"vgb1_{hh}", name=f"vgb1_{hh}")
                    for hh in range(2)]
            for hh in range(2):
                nc.scalar.dma_start(w2c[hh][:], w2c_d[hh, :, :])
                nc.scalar.dma_start(b1[hh][:], b1t_d[hh, :, :])
                nc.gpsimd.dma_start(vgw2[hh][:], vgw2t_d[hh, :, :])
                nc.gpsimd.dma_start(vgb1[hh][:], vgb1t_d[hh, :, :])
            onesb2 = cpool.tile([1, 133], BF16, tag="onesb2")
            nc.gpsimd.dma_start(onesb2[:], onesb2_d[0, :][None, :])
            onesp = cpool.tile([P, 1], BF16, tag="onesp")
            nc.scalar.dma_start(onesp[:], onesp_d[:, :])

            rel = cpool.tile([P, ET * 3], F32, tag="rel")
            nc.scalar.dma_start(rel[:], rel_d[:, :])
            velg = cpool.tile([P, NT * 15], F32, tag="velg")
            nc.scalar.dma_start(velg[:], velg_d[:, :])

            # all node features resident in SBUF: one DMA, no per-group loads
            hTall = cpool.tile([P, NT * 2 * P], BF16, tag="hTall")
            nc.scalar.dma_start(
                hTall[:].rearrange("p (t c) -> p t c", t=NT),
                hT[:, :, :].rearrange("t p c -> p t c"))

            # packed outputs; single DMA each at the end
            outbuf = cpool.tile([P, NT * 3], F32, tag="outbuf")
            geomb = cpool.tile([3, NT * P], F32, tag="geomb")

            relv = rel.rearrange("p (t f) -> p t f", f=3)
            rhv = hTall.rearrange("p (t kk n) -> p t kk n", kk=2, n=P)

            def node_group(g0: int, T: int):
                """Node tiles g0..g0+T-1: vel-gate MLP + vel combine."""
                psn = [ps1_pool.tile([P, NG * P], F32, tag="ps_mm1", name="psn")
                       for _ in range(2)]
                n_nch = (T * P + 511) // 512
                for hh in range(2):
                    for ch in range(n_nch):
                        tl, th = ch * 4, min(T, ch * 4 + 4)
                        for kk in range(2):
                            nc.tensor.matmul(
                                psn[hh][:, tl * P:th * P],
                                vgw1[kk][hh][:],
                                rhv[:, g0 + tl:g0 + th, kk, :],
                                start=(kk == 0), stop=(kk == 1))
                s1n = [nd_pool.tile([P, NG * P], BF16, tag=f"s1n_{hh}",
                                    name=f"s1n_{hh}")
                       for hh in range(2)]
                for hh in range(2):
                    nc.scalar.activation(s1n[hh][:, 0:T * P],
                                         psn[hh][:, 0:T * P], AF_USED,
                                         bias=vgb1[hh][:, 0:1], scale=1.0)
                for t in range(T):
                    nt = g0 + t
                    psa = psw_pool.tile([P, 8], F32, tag="ps_w", name="psa")
                    for hh in range(2):
                        nc.tensor.matmul(psa[:, 0:5],
                                         s1n[hh][:, t * P:(t + 1) * P],
                                         vgw2[hh][:],
                                         start=(hh == 0), stop=False)
                    nc.tensor.matmul(psa[:, 0:5], onesb2[:, 0:128],
                                     onesb2[:, 128:133], start=False, stop=True)

                    # out[:, j] = sum_k alpha[k] * vel[j, k]
                    scratch = sm_pool.tile([P, 15], F32, tag="scratch")
                    vbase = nt * 15
                    velg_v = velg[:, vbase:vbase + 15].rearrange(
                        "p (j k) -> p j k", k=5)
                    nc.vector.tensor_tensor(
                        scratch[:].rearrange("p (j k) -> p j k", k=5),
                        velg_v,
                        psa[:, None, 0:5].broadcast_to([P, 3, 5]),
                        op=OP.mult)
                    nc.vector.tensor_reduce(
                        outbuf[:, nt * 3:nt * 3 + 3, None],
                        scratch[:].rearrange("p (j k) -> p j k", k=5),
                        axis=mybir.AxisListType.X, op=OP.add)

            # ---- software-pipelined edge-path macro loop ------------------
            # PE program order per iteration m:
            #   mm1(m), u-mm(m-1), scatter(m-2)
            # so the PE never waits on the scalar/DVE chain of the current
            # macro and stays continuously busy (HAM stays at K=8/8).
            w1v = [w1[hh].rearrange("p (kk m) -> p kk m", kk=2)
                   for hh in range(2)]
            sup = None
            ohs = None
            stA = {}   # m -> state for u-mm+msg stage
            stB = {}   # m -> state for scatter stage
            geo_ref = [None]

            def head(m):
                nonlocal sup, ohs
                t0 = m * 8
                G = min(8, ET - t0)          # real edge tiles in this macro
                W = G * P                    # macro width in edges
                si, sm = divmod(m, 2)
                if sm == 0:
                    sup = rhs_pool.tile([P, 4096], F8, tag="sup", name="sup")
                    nc.sync.dma_start(sup[:], mijT[si, :, :])
                    ohs = oh_pool.tile([P, 2048], F8, tag="oh", name="ohs")
                    nc.sync.dma_start(ohs[:], ohT[si, :, :])
                supv = sup[:, sm * 2048:(sm + 1) * 2048].rearrange(
                    "p (kk e) -> p kk e", kk=2)
                ps1 = [ps1_pool.tile([P, 1024], F32, tag="ps_mm1", name="ps1")
                       for _ in range(2)]
                n_ch = (W + 511) // 512
                for hh in range(2):
                    for ch in range(n_ch):
                        cw = min(512, W - ch * 512)
                        nc.tensor.matmul(
                            ps1[hh][:, ch * 512:ch * 512 + cw],
                            w1v[hh],
                            supv[:, :, ch * 512:ch * 512 + cw],
                            start=True, stop=True, perf_mode=DR)
                s1 = [s1_pool.tile([P, 1024], BF16, tag=f"s1_{hh}",
                                   name=f"s1_{hh}")
                      for hh in range(2)]
                for hh in range(2):
                    nc.scalar.activation(s1[hh][:, 0:W], ps1[hh][:, 0:W],
                                         AF_USED,
                                         bias=b1[hh][:, 0:1],
                                         scale=1.0 / (MM_SCALE * MM_SCALE))
                # u1 = s1_0*w2_0 + s1_1*w2_1  (DVE: one TS + one STT)
                t1 = s1_pool.tile([P, 1024], BF16, tag="t1", name="t1")
                nc.vector.tensor_scalar(t1[:, 0:W], s1[1][:, 0:W],
                                        w2c[1][:, 0:1], None, op0=OP.mult)
                u1 = s1_pool.tile([P, 1024], BF16, tag="u1", name="u1")
                nc.vector.scalar_tensor_tensor(
                    u1[:, 0:W], s1[0][:, 0:W], w2c[0][:, 0:1], t1[:, 0:W],
                    op0=OP.mult, op1=OP.add)
                stA[m] = (u1, G, t0)

            def stage_umm(m):
                u1, G, t0 = stA.pop(m)
                # per-tile w sums: psw[:, c] = sum_h u1[h, tile c]
                psw = psw_pool.tile([P, 8], F32, tag="ps_w")
                for c in range(G):
                    nc.tensor.matmul(psw[:, c:c + 1],
                                     u1[:, c * P:(c + 1) * P],
                                     onesp[:], start=True, stop=True)
                # msg for all tiles in one op: [P, G, 3] = (w + b2) * rel
                msg = sm_pool.tile([P, 24], BF16, tag="msg")
                nc.vector.scalar_tensor_tensor(
                    msg[:, 0:3 * G].rearrange("p (c f) -> p c f", f=3),
                    psw[:, 0:G, None].broadcast_to([P, G, 3]),
                    float(b2),
                    relv[:, t0:t0 + G, 0:3],
                    op0=OP.add, op1=OP.mult)
                stB[m] = msg

            def stage_scatter(m, ohs_m):
                msg = stB.pop(m)
                t0 = m * 8
                G = min(8, ET - t0)
                sm = m % 2
                for c in range(G):
                    t = t0 + c
                    j, i, K = slot_of[t]
                    if j % 4 == 0 and i == 0:
                        geo_ref[0] = psg_pool.tile(
                            [3, 512], F32, tag="ps_geo", name="geo")
                    ohc = ohs_m[:, sm * 1024 + c * P:sm * 1024 + (c + 1) * P]
                    off = (j % 4) * P
                    nc.tensor.matmul(geo_ref[0][0:3, off:off + P],
                                     msg[:, 3 * c:3 * c + 3],
                                     ohc,
                                     start=(i == 0), stop=(i == K - 1))
                    if i == K - 1 and (j % 4 == 3 or j == NT - 1):
                        lo = (j // 4) * 512
                        wdt = (j % 4) * P + P
                        nc.vector.tensor_copy(geomb[0:3, lo:lo + wdt],
                                              geo_ref[0][0:3, 0:wdt])

            # node groups spread evenly through the macro loop
            n_grp = (NT + NG - 1) // NG
            trig = {max(1, ((idx + 1) * (n_mac + 2)) // (n_grp + 1)): idx
                    for idx in range(n_grp)}

            oh_hist = {}
            for m in range(n_mac + 2):
                if m < n_mac:
                    head(m)
                    oh_hist[m] = ohs
                if m >= 1 and m - 1 < n_mac:
                    stage_umm(m - 1)
                if m >= 2:
                    stage_scatter(m - 2, oh_hist.pop(m - 2))
                if m in trig:
                    g0 = trig[m] * NG
                    node_group(g0, min(NG, NT - g0))

            nc.sync.dma_start(out_d[:, :], outbuf[:])
            nc.sync.dma_start(geo_d[:, :], geomb[0:3, :])

    _split_excess_waits(nc)
    return nc


def _preprocess(inputs: dict):
    """Shard + lay out all per-core device inputs.

    Returns (in_maps, Ks, b2, perms, invc)."""
    h = np.asarray(inputs["h"], np.float32)
    m_ij = np.asarray(inputs["m_ij"], np.float32)
    x = np.asarray(inputs["x"], np.float32)
    vel_all = np.asarray(inputs["vel_all"], np.float32)
    ei = np.asarray(inputs["edge_index"])
    src = ei[0].astype(np.int64)
    dst = ei[1].astype(np.int64)

    counts = np.bincount(dst, minlength=N_NODES).astype(np.float32)
    invc = (1.0 / np.maximum(counts, 1.0)).astype(np.float32)

    order = np.argsort(dst, kind="stable")
    dst_s = dst[order]
    src_s = src[order]
    rel_s = (x[src_s] - x[dst_s]).astype(np.float32)
    mij_s = (m_ij[order] * MM_SCALE).astype(F8_NP)

    g_all = dst_s // P                   # global 128-node group, 0..391
    n_groups = N_CORES * NT
    cg = np.bincount(g_all, minlength=n_groups)
    gstart = np.zeros(n_groups + 1, np.int64)
    gstart[1:] = np.cumsum(cg)
    tiles_needed = -(-cg // P)           # ceil, [392]
    tn = tiles_needed.reshape(N_CORES, NT)

    # schedule: per core, sort its groups by tile count desc; slot j's
    # size is the max over cores (shared SPMD program structure).
    perms = [np.argsort(-tn[k], kind="stable") for k in range(N_CORES)]
    Ks = np.max(np.stack([tn[k][perms[k]] for k in range(N_CORES)]), axis=0)
    Ks = Ks.astype(np.int64)
    Ks = np.maximum(Ks, 1)
    ET = int(Ks.sum())
    pad = (-ET) % 8
    Ks[-1] += pad                        # pad tiles ride in the last slot
    ET += pad
    Ks = [int(v) for v in Ks]
    kstart = np.zeros(NT + 1, np.int64)
    kstart[1:] = np.cumsum(Ks)

    # weights (shared by all cores)
    w1 = np.asarray(inputs["ew_W1"], np.float32)
    b1 = np.asarray(inputs["ew_b1"], np.float32)
    w2 = np.asarray(inputs["ew_W2"], np.float32)
    b2 = float(np.asarray(inputs["ew_b2"], np.float32)[0])
    vgw1 = np.asarray(inputs["vg_W1"], np.float32)
    vgb1 = np.asarray(inputs["vg_b1"], np.float32)
    vgw2 = np.asarray(inputs["vg_W2"], np.float32)
    vgb2 = np.asarray(inputs["vg_b2"], np.float32)

    # w1dr[hh][p, kk*128+m] = 16*W1[kk*128+p, hh*128+m]  (DoubleRow layout)
    w1s = (w1 * MM_SCALE).reshape(2, P, 2, P)            # [kk, p, hh, m]
    w1dr = np.ascontiguousarray(
        w1s.transpose(2, 1, 0, 3)).reshape(2, P, 2 * P).astype(F8_NP)
    w2c = w2.reshape(2, P, 1).astype(np.float32).copy()
    b1t = b1.reshape(2, P, 1).copy()
    vgw1b = vgw1.reshape(2, P, 2, P).transpose(0, 2, 1, 3).astype(BF16_NP).copy()
    vgw2t = vgw2.reshape(2, P, 5).astype(BF16_NP).copy()
    vgb1t = vgb1.reshape(2, P, 1).copy()
    onesb2 = np.zeros((1, 133), BF16_NP)
    onesb2[0, :P] = 1.0
    onesb2[0, P:P + 5] = vgb2.astype(BF16_NP)
    onesp = np.ones((P, 1), BF16_NP)

    # padded node tensors
    hp = np.zeros((N_PAD, H), np.float32)
    hp[:N_NODES] = h
    velp = np.zeros((N_PAD, 5, 3), np.float32)
    velp[:N_NODES] = vel_all

    n_mac = ET // 8
    n_sup = (n_mac + 1) // 2
    S = ET * P                           # edge slots per core

    in_maps = []
    for k in range(N_CORES):
        perm = perms[k]
        # gather this core's edges into slot order
        mijp = np.zeros((S, H), F8_NP)
        relp = np.zeros((S, 3), np.float32)
        colidx = np.full(S, -1, np.int64)
        for j in range(NT):
            g_local = int(perm[j])
            g = k * NT + g_local
            e0, e1 = int(gstart[g]), int(gstart[g + 1])
            s0 = int(kstart[j]) * P
            mijp[s0:s0 + e1 - e0] = mij_s[e0:e1]
            relp[s0:s0 + e1 - e0] = rel_s[e0:e1]
            colidx[s0:s0 + e1 - e0] = dst_s[e0:e1] - (g * P)

        mv = mijp.reshape(ET, P, H)
        rv = relp.reshape(ET, P, 3)
        cv = colidx.reshape(ET, P)

        # mijT supertiles: [n_sup, 128, 4096]; free = (mac, kk, tile, e),
        # partition = h within kk half
        b = mv.transpose(0, 2, 1).reshape(ET, 2, P, P)
        full = np.zeros((n_sup * 16, 2, P, P), F8_NP)
        full[:ET] = b
        mijT = np.ascontiguousarray(
            full.reshape(n_sup, 2, 8, 2, P, P).transpose(0, 4, 1, 3, 2, 5)
        ).reshape(n_sup, P, 4096)

        rel = np.ascontiguousarray(
            rv.transpose(1, 0, 2)).reshape(P, ET * 3)

        # host-built one-hot: ohT[sup][p, (tile16, n)] = (col[t, p] == n)
        oh = (cv[:, :, None] ==
              np.arange(P, dtype=np.int64)[None, None, :])  # [ET, P, 128]
        ohfull = np.zeros((n_sup * 16, P, P), F8_NP)
        ohfull[:ET] = oh.astype(F8_NP)
        ohT = np.ascontiguousarray(
            ohfull.reshape(n_sup, 16, P, P).transpose(0, 2, 1, 3)
        ).reshape(n_sup, P, 2048)

        hk = hp[k * NPC:(k + 1) * NPC].reshape(NT, P, H)
        hTk = np.ascontiguousarray(
            hk.transpose(0, 2, 1).reshape(NT, 2, P, P).transpose(0, 2, 1, 3)
        ).reshape(NT, P, 2 * P).astype(BF16_NP)

        # velg cols per node tile: [comp j, gate k] at 5*j+k
        vg = (velp[k * NPC:(k + 1) * NPC]
              .reshape(NT, P, 5, 3).transpose(1, 0, 3, 2)
              .reshape(P, NT * 15))
        velg = np.ascontiguousarray(vg)

        in_maps.append({
            "mijT": mijT,
            "ohT": ohT,
            "rel": rel,
            "hT": hTk,
            "velg": velg,
            "w1dr": w1dr,
            "w2c": w2c,
            "b1t": b1t,
            "vgw1b": vgw1b,
            "vgw2t": vgw2t,
            "vgb1t": vgb1t,
            "onesb2": onesb2,
            "onesp": onesp,
        })
    return in_maps, Ks, b2, perms, invc


def unpack_out(arr: np.ndarray) -> np.ndarray:
    """[128, NT*3] packed per-core output -> [NPC, 3]."""
    return arr.reshape(P, NT, 3).transpose(1, 0, 2).reshape(NPC, 3)


def finalize(results, perms, invc) -> np.ndarray:
    """Combine per-core vel output + slot-ordered geom sums on the host."""
    out = np.empty((N_PAD, 3), np.float32)
    for k in range(N_CORES):
        vel = unpack_out(np.asarray(results[k]["out"], np.float32))
        geo = np.asarray(results[k]["geo"], np.float32)  # [3, NT*128]
        geoT = geo.T.reshape(NT, P, 3)
        geom = np.empty((NPC, 3), np.float32)
        perm = perms[k]
        for j in range(NT):
            geom[perm[j] * P:(perm[j] + 1) * P] = geoT[j]
        nodes = slice(k * NPC, (k + 1) * NPC)
        iv = np.ones(NPC, np.float32)
        n_real = min(N_NODES - k * NPC, NPC)
        if n_real > 0:
            iv[:n_real] = invc[k * NPC:k * NPC + n_real]
        out[nodes] = vel + geom * iv[:, None]
    return out[:N_NODES]


def kernel(**inputs) -> np.ndarray:
    in_maps, Ks, b2, perms, invc = _preprocess(inputs)
    nc = _build_program(Ks, b2)
    res = run_bass_kernel_spmd(nc, in_maps, list(range(N_CORES)))
    return finalize(res.results, perms, invc).astype(np.float32)


# revision 7
# speedup vs baseline: 1.1598x; 1.1585x over previous
"""Trainium2 Bass kernel for nn_EquivariantDecoder (GNN message passing).

Sharding: nodes are split into 8 contiguous ranges of 6272 (= 49 tiles of
128); each core owns the edges whose dst lands in its range, so per-node
segment sums are core-local (no collectives).

v2 layout: each core sorts its 49 node-groups by edge-tile count
descending; schedule slot j gets K_j = max over cores of the j-th largest
per-group tile count, so one SPMD program covers all cores with ~2% edge
padding (host un-permutes the per-slot geom output at the end).

Device work per core:
  edge path:  z = m_ij @ W1 via fp8(e4m3) DoubleRow matmuls (inputs
              pre-scaled x16 on host, rescaled by the activation's 1/256)
              s1 = silu(z + b1)                     (scalar engine, bf16)
              t1 = s1_1*w2_1 (DVE TS); u1 = s1_0*w2_0 + t1 (DVE STT)
              psw = colsum(u1) via ones-rhs matmuls (PE, 1 per tile)
              msg = (psw + b2) * rel                (DVE STT, bf16)
              scatter: flipped one-hot matmul: lhsT = msg [128e, 3],
              rhs = host-built one-hot [128e, 128n] fp8 -> psum [3, 128n]
              accumulated per slot; banks of 4 slots copied to SBUF.
  node path:  alpha = silu(h @ vgW1 + vgb1) @ vgW2 + vgb2, batched 8
              node tiles per group; out = sum_k alpha_k * vel_k
              (geom mean + add folded into the host finalize).
"""

import sys

import numpy as np

try:
    import concourse.bass as bass  # noqa: F401
except Exception:  # pragma: no cover
    sys.path.insert(0, "/opt/trn_rl_repo")

import concourse.bass as bass
import concourse.mybir as mybir
from concourse.bass_utils import run_bass_kernel_spmd
from concourse.tile import TileContext
from concourse.vector_clock import ScopedClock

N_NODES = 50000
N_EDGES = 800000
H = 256
N_CORES = 8
NT = 49                 # node tiles (=groups) per core
NPC = NT * 128          # 6272 nodes per core
N_PAD = N_CORES * NPC   # 50176
P = 128
NG = 8                  # node tiles batched per node-path group

BF16 = mybir.dt.bfloat16
BF16_NP = mybir.dt.np(BF16)
F8 = mybir.dt.float8e4
F8_NP = mybir.dt.np(F8)
F32 = mybir.dt.float32
AF = mybir.ActivationFunctionType
AF_USED = AF.Silu   # sim tests may override (interp lacks Silu)
OP = mybir.AluOpType
DR = mybir.MatmulPerfMode.DoubleRow
MM_SCALE = 16.0         # host pre-scale on m_ij and W1 (undone by 1/256)


# ---------------------------------------------------------------------------
# Walrus on this toolchain rejects >2 sync waits on the TileContext tail
# drain ("Too many sync wait commands"); split them across SP NOPs.
def _patched_drain_and_barrier(self, tick_clock, wait_clock):
    drain_inst = self.nc.sync.drain()
    wait_clock.add_sem_waits(
        drain_inst.ins, ScopedClock({None: tick_clock.global_clock})
    )
    si = drain_inst.ins.sync_info
    if si is not None and si.on_wait and len(si.on_wait) > 1:
        extra = list(si.on_wait[1:])
        del si.on_wait[1:]
        for w in extra:
            nop = self.nc.sync.nop(nofuse=True, hint="drain_wait_split")
            nsi = nop.ins.sync_info
            if nsi is None:
                nop.ins.sync_info = mybir.SyncInfo(on_wait=[w], on_update=[])
            else:
                nsi.on_wait.append(w)

    self.nc.all_engine_barrier()
    assert self.sems is not None
    popped = self.nc._tile_sem_poison_stack.pop()
    assert popped is self._sem_poison
    self.nc.clear_and_free_semaphores(list(self.sems.allocated().values()))
    self.nc.all_engine_barrier()


TileContext._drain_and_barrier = _patched_drain_and_barrier


def _split_excess_waits(nc, maxw: int = 1):
    """Walrus rejects >maxw sync waits on one instruction; move the excess
    onto NOPs inserted just before, on the same engine (same-queue program
    order makes this equivalent)."""
    n_split = 0
    for f in nc.m.functions:
        for b in f.blocks:
            out = []
            for inst in b.instructions:
                si = inst.sync_info
                if si is not None and si.on_wait and len(si.on_wait) > maxw:
                    extra = list(si.on_wait[: -maxw])
                    del si.on_wait[: -maxw]
                    for i in range(0, len(extra), maxw):
                        nop = mybir.InstNoOp(
                            name=f"{inst.name}-wsplit{i}",
                            engine=inst.engine,
                            sync_info=mybir.SyncInfo(
                                on_wait=extra[i:i + maxw], on_update=[]),
                            bass_nofuse=True,
                        )
                        out.append(nop)
                    n_split += 1
                out.append(inst)
            b.instructions[:] = out
    return n_split
# ---------------------------------------------------------------------------


def _build_program(Ks: list[int], b2: float):
    """Trace the single-core SPMD program. Ks[j] = edge tiles in schedule
    slot j (shared across cores)."""
    ET = sum(Ks)                     # edge tiles per core (multiple of 8)
    n_mac = ET // 8                  # macros of 8 edge tiles
    n_sup = (n_mac + 1) // 2         # supertiles of 2 macros (1 DMA each)

    # tile -> (slot, idx-in-slot, slot-size)
    slot_of = []
    for j, K in enumerate(Ks):
        for i in range(K):
            slot_of.append((j, i, K))

    nc = bass.Bass()

    mijT = nc.dram_tensor("mijT", [n_sup, P, 4096], F8, kind="ExternalInput")
    ohT = nc.dram_tensor("ohT", [n_sup, P, 2048], F8, kind="ExternalInput")
    rel_d = nc.dram_tensor("rel", [P, ET * 3], F32, kind="ExternalInput")
    hT = nc.dram_tensor("hT", [NT, P, 2 * P], BF16, kind="ExternalInput")
    velg_d = nc.dram_tensor("velg", [P, NT * 15], F32, kind="ExternalInput")
    w1dr_d = nc.dram_tensor("w1dr", [2, P, 2 * P], F8, kind="ExternalInput")
    w2c_d = nc.dram_tensor("w2c", [2, P, 1], F32, kind="ExternalInput")
    b1t_d = nc.dram_tensor("b1t", [2, P, 1], F32, kind="ExternalInput")
    vgw1b_d = nc.dram_tensor("vgw1b", [2, 2, P, P], BF16, kind="ExternalInput")
    vgw2t_d = nc.dram_tensor("vgw2t", [2, P, 5], BF16, kind="ExternalInput")
    vgb1t_d = nc.dram_tensor("vgb1t", [2, P, 1], F32, kind="ExternalInput")
    onesb2_d = nc.dram_tensor("onesb2", [1, 133], BF16, kind="ExternalInput")
    onesp_d = nc.dram_tensor("onesp", [P, 1], BF16, kind="ExternalInput")
    out_d = nc.dram_tensor("out", [P, NT * 3], F32, kind="ExternalOutput")
    geo_d = nc.dram_tensor("geo", [3, NT * P], F32, kind="ExternalOutput")

    with TileContext(nc) as tc:
        with (
            tc.tile_pool(name="const", bufs=1) as cpool,
            tc.tile_pool(name="rhs", bufs=4) as rhs_pool,
            tc.tile_pool(name="s1", bufs=4) as s1_pool,
            tc.tile_pool(name="small", bufs=6) as sm_pool,
            tc.tile_pool(name="oh", bufs=6) as oh_pool,
            tc.tile_pool(name="nodes", bufs=3) as nd_pool,
            tc.tile_pool(name="ps_mm1", bufs=3, space="PSUM") as ps1_pool,
            tc.tile_pool(name="ps_w", bufs=1, space="PSUM") as psw_pool,
            tc.tile_pool(name="ps_geo", bufs=1, space="PSUM") as psg_pool,
        ):
            # ---- edge-path streaming prefetch (issued FIRST so the first
            # macros' mm1 inputs are not queued behind the big const DMAs)
            sup_t = {}
            oh_t = {}

            def prefetch(si):
                st = rhs_pool.tile([P, 4096], F8, tag="sup", name="sup")
                nc.sync.dma_start(st[:], mijT[si, :, :])
                ot = oh_pool.tile([P, 2048], F8, tag="oh", name="ohs")
                nc.sync.dma_start(ot[:], ohT[si, :, :])
                sup_t[si] = st
                oh_t[si] = ot

            w1 = [cpool.tile([P, 2 * P], F8, tag=f"w1_{hh}", name=f"w1_{hh}")
                  for hh in range(2)]
            for hh in range(2):
                nc.sync.dma_start(w1[hh][:], w1dr_d[hh, :, :])
            for si in range(min(2, n_sup)):
                prefetch(si)

            vgw1 = [[cpool.tile([P, P], BF16, tag=f"vgw1_{kk}{hh}",
                                name=f"vgw1_{kk}{hh}")
                     for hh in range(2)] for kk in range(2)]
            for kk in range(2):
                for hh in range(2):
                    nc.gpsimd.dma_start(vgw1[kk][hh][:], vgw1b_d[kk, hh, :, :])
            w2c = [cpool.tile([P, 1], F32, tag=f"w2c_{hh}", name=f"w2c_{hh}")
                   for hh in range(2)]
            b1 = [cpool.tile([P, 1], F32, tag=f"b1_{hh}", name=f"b1_{hh}")
                  for hh in range(2)]
            vgw2 = [cpool.tile([P, 5], BF16, tag=f"vgw2_{hh}", name=f"vgw2_{hh}")
                    for hh in range(2)]
            vgb1 = [cpool.tile([P, 1], F32, tag=f"vgb1_{hh}", name=f"vgb1_{hh}")
                    for hh in range(2)]
            for hh in range(2):
                nc.scalar.dma_start(w2c[hh][:], w2c_d[hh, :, :])
                nc.scalar.dma_start(b1[hh][:], b1t_d[hh, :, :])
                nc.gpsimd.dma_start(vgw2[hh][:], vgw2t_d[hh, :, :])
                nc.gpsimd.dma_start(vgb1[hh][:], vgb1t_d[hh, :, :])
            onesb2 = cpool.tile([1, 133], BF16, tag="onesb2")
            nc.gpsimd.dma_start(onesb2[:], onesb2_d[0, :][None, :])
            onesp = cpool.tile([P, 1], BF16, tag="onesp")
            nc.scalar.dma_start(onesp[:], onesp_d[:, :])

            rel = cpool.tile([P, ET * 3], F32, tag="rel")
            nc.scalar.dma_start(rel[:], rel_d[:, :])
            velg = cpool.tile([P, NT * 15], F32, tag="velg")
            nc.gpsimd.dma_start(velg[:], velg_d[:, :])

            # all node features resident in SBUF: one DMA, no per-group loads
            hTall = cpool.tile([P, NT * 2 * P], BF16, tag="hTall")
            nc.gpsimd.dma_start(
                hTall[:].rearrange("p (t c) -> p t c", t=NT),
                hT[:, :, :].rearrange("t p c -> p t c"))

            # packed outputs; single DMA each at the end
            outbuf = cpool.tile([P, NT * 3], F32, tag="outbuf")
            geomb = cpool.tile([3, NT * P], F32, tag="geomb")

            relv = rel.rearrange("p (t f) -> p t f", f=3)
            rhv = hTall.rearrange("p (t kk n) -> p t kk n", kk=2, n=P)

            def node_group(g0: int, T: int):
                """Node tiles g0..g0+T-1: vel-gate MLP + vel combine."""
                psn = [ps1_pool.tile([P, NG * P], F32, tag="ps_mm1", name="psn")
                       for _ in range(2)]
                n_nch = (T * P + 511) // 512
                for hh in range(2):
                    for ch in range(n_nch):
                        tl, th = ch * 4, min(T, ch * 4 + 4)
                        for kk in range(2):
                            nc.tensor.matmul(
                                psn[hh][:, tl * P:th * P],
                                vgw1[kk][hh][:],
                                rhv[:, g0 + tl:g0 + th, kk, :],
                                start=(kk == 0), stop=(kk == 1))
                s1n = [nd_pool.tile([P, NG * P], BF16, tag=f"s1n_{hh}",
                                    name=f"s1n_{hh}")
                       for hh in range(2)]
                for hh in range(2):
                    nc.scalar.activation(s1n[hh][:, 0:T * P],
                                         psn[hh][:, 0:T * P], AF_USED,
                                         bias=vgb1[hh][:, 0:1], scale=1.0)
                for t in range(T):
                    nt = g0 + t
                    psa = psw_pool.tile([P, 8], F32, tag="ps_w", name="psa")
                    for hh in range(2):
                        nc.tensor.matmul(psa[:, 0:5],
                                         s1n[hh][:, t * P:(t + 1) * P],
                                         vgw2[hh][:],
                                         start=(hh == 0), stop=False)
                    nc.tensor.matmul(psa[:, 0:5], onesb2[:, 0:128],
                                     onesb2[:, 128:133], start=False, stop=True)

                    # out[:, j] = sum_k alpha[k] * vel[j, k]
                    scratch = sm_pool.tile([P, 15], F32, tag="scratch")
                    vbase = nt * 15
                    velg_v = velg[:, vbase:vbase + 15].rearrange(
                        "p (j k) -> p j k", k=5)
                    nc.vector.tensor_tensor(
                        scratch[:].rearrange("p (j k) -> p j k", k=5),
                        velg_v,
                        psa[:, None, 0:5].broadcast_to([P, 3, 5]),
                        op=OP.mult)
                    nc.vector.tensor_reduce(
                        outbuf[:, nt * 3:nt * 3 + 3, None],
                        scratch[:].rearrange("p (j k) -> p j k", k=5),
                        axis=mybir.AxisListType.X, op=OP.add)

            # ---- software-pipelined edge-path macro loop ------------------
            # PE program order per iteration m:
            #   mm1(m), u-mm(m-1), scatter(m-2)
            # so the PE never waits on the scalar/DVE chain of the current
            # macro and stays continuously busy (HAM stays at K=8/8).
            w1v = [w1[hh].rearrange("p (kk m) -> p kk m", kk=2)
                   for hh in range(2)]
            sup = None
            ohs = None
            stA = {}   # m -> state for u-mm+msg stage
            stB = {}   # m -> state for scatter stage
            geo_ref = [None]

            def head(m):
                nonlocal sup, ohs
                t0 = m * 8
                G = min(8, ET - t0)          # real edge tiles in this macro
                W = G * P                    # macro width in edges
                si, sm = divmod(m, 2)
                if sm == 0:
                    if si + 2 < n_sup:
                        prefetch(si + 2)
                    sup = sup_t.pop(si)
                    ohs = oh_t.pop(si)
                supv = sup[:, sm * 2048:(sm + 1) * 2048].rearrange(
                    "p (kk e) -> p kk e", kk=2)
                ps1 = [ps1_pool.tile([P, 1024], F32, tag="ps_mm1", name="ps1")
                       for _ in range(2)]
                n_ch = (W + 511) // 512
                for hh in range(2):
                    for ch in range(n_ch):
                        cw = min(512, W - ch * 512)
                        nc.tensor.matmul(
                            ps1[hh][:, ch * 512:ch * 512 + cw],
                            w1v[hh],
                            supv[:, :, ch * 512:ch * 512 + cw],
                            start=True, stop=True, perf_mode=DR)
                s1 = [s1_pool.tile([P, 1024], BF16, tag=f"s1_{hh}",
                                   name=f"s1_{hh}")
                      for hh in range(2)]
                for hh in range(2):
                    nc.scalar.activation(s1[hh][:, 0:W], ps1[hh][:, 0:W],
                                         AF_USED,
                                         bias=b1[hh][:, 0:1],
                                         scale=1.0 / (MM_SCALE * MM_SCALE))
                # u = s1_0*w2_0 + s1_1*w2_1  (DVE; tensor_scalar runs 4x on
                # bf16 SBUF, tensor_tensor 2x -- scalar_tensor_tensor only
                # has a 1x uop, so the 3-op form is faster)
                u0 = s1_pool.tile([P, 1024], BF16, tag="u0", name="u0")
                nc.vector.tensor_scalar(u0[:, 0:W], s1[0][:, 0:W],
                                        w2c[0][:, 0:1], None, op0=OP.mult)
                t1 = s1_pool.tile([P, 1024], BF16, tag="t1", name="t1")
                nc.vector.tensor_scalar(t1[:, 0:W], s1[1][:, 0:W],
                                        w2c[1][:, 0:1], None, op0=OP.mult)
                u1 = s1_pool.tile([P, 1024], BF16, tag="u1", name="u1")
                nc.vector.tensor_add(u1[:, 0:W], t1[:, 0:W], u0[:, 0:W])
                stA[m] = (u1, G, t0)

            def stage_umm(m):
                u1, G, t0 = stA.pop(m)
                # per-tile w sums: psw[:, c] = sum_h u1[h, tile c]
                psw = psw_pool.tile([P, 8], F32, tag="ps_w")
                for c in range(G):
                    nc.tensor.matmul(psw[:, c:c + 1],
                                     u1[:, c * P:(c + 1) * P],
                                     onesp[:], start=True, stop=True)
                # msg for all tiles in one op: [P, G, 3] = (w + b2) * rel
                msg = sm_pool.tile([P, 24], BF16, tag="msg")
                nc.vector.scalar_tensor_tensor(
                    msg[:, 0:3 * G].rearrange("p (c f) -> p c f", f=3),
                    psw[:, 0:G, None].broadcast_to([P, G, 3]),
                    float(b2),
                    relv[:, t0:t0 + G, 0:3],
                    op0=OP.add, op1=OP.mult)
                stB[m] = msg

            def stage_scatter(m, ohs_m):
                msg = stB.pop(m)
                t0 = m * 8
                G = min(8, ET - t0)
                sm = m % 2
                for c in range(G):
                    t = t0 + c
                    j, i, K = slot_of[t]
                    if j % 4 == 0 and i == 0:
                        geo_ref[0] = psg_pool.tile(
                            [3, 512], F32, tag="ps_geo", name="geo")
                    ohc = ohs_m[:, sm * 1024 + c * P:sm * 1024 + (c + 1) * P]
                    off = (j % 4) * P
                    nc.tensor.matmul(geo_ref[0][0:3, off:off + P],
                                     msg[:, 3 * c:3 * c + 3],
                                     ohc,
                                     start=(i == 0), stop=(i == K - 1))
                    if i == K - 1 and (j % 4 == 3 or j == NT - 1):
                        lo = (j // 4) * 512
                        wdt = (j % 4) * P + P
                        nc.vector.tensor_copy(geomb[0:3, lo:lo + wdt],
                                              geo_ref[0][0:3, 0:wdt])

            # node groups spread evenly through the macro loop
            n_grp = (NT + NG - 1) // NG
            trig = {max(1, ((idx + 1) * (n_mac + 2)) // (n_grp + 1)): idx
                    for idx in range(n_grp)}

            oh_hist = {}
            for m in range(n_mac + 2):
                if m < n_mac:
                    head(m)
                    oh_hist[m] = ohs
                if m >= 1 and m - 1 < n_mac:
                    stage_umm(m - 1)
                if m >= 2:
                    stage_scatter(m - 2, oh_hist.pop(m - 2))
                if m in trig:
                    g0 = trig[m] * NG
                    node_group(g0, min(NG, NT - g0))

            nc.sync.dma_start(out_d[:, :], outbuf[:])
            nc.sync.dma_start(geo_d[:, :], geomb[0:3, :])

    _split_excess_waits(nc)
    return nc


def _preprocess(inputs: dict):
    """Shard + lay out all per-core device inputs.

    Returns (in_maps, Ks, b2, perms, invc)."""
    h = np.asarray(inputs["h"], np.float32)
    m_ij = np.asarray(inputs["m_ij"], np.float32)
    x = np.asarray(inputs["x"], np.float32)
    vel_all = np.asarray(inputs["vel_all"], np.float32)
    ei = np.asarray(inputs["edge_index"])
    src = ei[0].astype(np.int64)
    dst = ei[1].astype(np.int64)

    counts = np.bincount(dst, minlength=N_NODES).astype(np.float32)
    invc = (1.0 / np.maximum(counts, 1.0)).astype(np.float32)

    order = np.argsort(dst, kind="stable")
    dst_s = dst[order]
    src_s = src[order]
    rel_s = (x[src_s] - x[dst_s]).astype(np.float32)
    mij_s = (m_ij[order] * MM_SCALE).astype(F8_NP)

    g_all = dst_s // P                   # global 128-node group, 0..391
    n_groups = N_CORES * NT
    cg = np.bincount(g_all, minlength=n_groups)
    gstart = np.zeros(n_groups + 1, np.int64)
    gstart[1:] = np.cumsum(cg)
    tiles_needed = -(-cg // P)           # ceil, [392]
    tn = tiles_needed.reshape(N_CORES, NT)

    # schedule: per core, sort its groups by tile count desc; slot j's
    # size is the max over cores (shared SPMD program structure).
    perms = [np.argsort(-tn[k], kind="stable") for k in range(N_CORES)]
    Ks = np.max(np.stack([tn[k][perms[k]] for k in range(N_CORES)]), axis=0)
    Ks = Ks.astype(np.int64)
    Ks = np.maximum(Ks, 1)
    ET = int(Ks.sum())
    pad = (-ET) % 8
    Ks[-1] += pad                        # pad tiles ride in the last slot
    ET += pad
    Ks = [int(v) for v in Ks]
    kstart = np.zeros(NT + 1, np.int64)
    kstart[1:] = np.cumsum(Ks)

    # weights (shared by all cores)
    w1 = np.asarray(inputs["ew_W1"], np.float32)
    b1 = np.asarray(inputs["ew_b1"], np.float32)
    w2 = np.asarray(inputs["ew_W2"], np.float32)
    b2 = float(np.asarray(inputs["ew_b2"], np.float32)[0])
    vgw1 = np.asarray(inputs["vg_W1"], np.float32)
    vgb1 = np.asarray(inputs["vg_b1"], np.float32)
    vgw2 = np.asarray(inputs["vg_W2"], np.float32)
    vgb2 = np.asarray(inputs["vg_b2"], np.float32)

    # w1dr[hh][p, kk*128+m] = 16*W1[kk*128+p, hh*128+m]  (DoubleRow layout)
    w1s = (w1 * MM_SCALE).reshape(2, P, 2, P)            # [kk, p, hh, m]
    w1dr = np.ascontiguousarray(
        w1s.transpose(2, 1, 0, 3)).reshape(2, P, 2 * P).astype(F8_NP)
    w2c = w2.reshape(2, P, 1).astype(np.float32).copy()
    b1t = b1.reshape(2, P, 1).copy()
    vgw1b = vgw1.reshape(2, P, 2, P).transpose(0, 2, 1, 3).astype(BF16_NP).copy()
    vgw2t = vgw2.reshape(2, P, 5).astype(BF16_NP).copy()
    vgb1t = vgb1.reshape(2, P, 1).copy()
    onesb2 = np.zeros((1, 133), BF16_NP)
    onesb2[0, :P] = 1.0
    onesb2[0, P:P + 5] = vgb2.astype(BF16_NP)
    onesp = np.ones((P, 1), BF16_NP)

    # padded node tensors
    hp = np.zeros((N_PAD, H), np.float32)
    hp[:N_NODES] = h
    velp = np.zeros((N_PAD, 5, 3), np.float32)
    velp[:N_NODES] = vel_all

    n_mac = ET // 8
    n_sup = (n_mac + 1) // 2
    S = ET * P                           # edge slots per core

    in_maps = []
    for k in range(N_CORES):
        perm = perms[k]
        # gather this core's edges into slot order
        mijp = np.zeros((S, H), F8_NP)
        relp = np.zeros((S, 3), np.float32)
        colidx = np.full(S, -1, np.int64)
        for j in range(NT):
            g_local = int(perm[j])
            g = k * NT + g_local
            e0, e1 = int(gstart[g]), int(gstart[g + 1])
            s0 = int(kstart[j]) * P
            mijp[s0:s0 + e1 - e0] = mij_s[e0:e1]
            relp[s0:s0 + e1 - e0] = rel_s[e0:e1]
            colidx[s0:s0 + e1 - e0] = dst_s[e0:e1] - (g * P)

        mv = mijp.reshape(ET, P, H)
        rv = relp.reshape(ET, P, 3)
        cv = colidx.reshape(ET, P)

        # mijT supertiles: [n_sup, 128, 4096]; free = (mac, kk, tile, e),
        # partition = h within kk half
        b = mv.transpose(0, 2, 1).reshape(ET, 2, P, P)
        full = np.zeros((n_sup * 16, 2, P, P), F8_NP)
        full[:ET] = b
        mijT = np.ascontiguousarray(
            full.reshape(n_sup, 2, 8, 2, P, P).transpose(0, 4, 1, 3, 2, 5)
        ).reshape(n_sup, P, 4096)

        rel = np.ascontiguousarray(
            rv.transpose(1, 0, 2)).reshape(P, ET * 3)

        # host-built one-hot: ohT[sup][p, (tile16, n)] = (col[t, p] == n)
        oh = (cv[:, :, None] ==
              np.arange(P, dtype=np.int64)[None, None, :])  # [ET, P, 128]
        ohfull = np.zeros((n_sup * 16, P, P), F8_NP)
        ohfull[:ET] = oh.astype(F8_NP)
        ohT = np.ascontiguousarray(
            ohfull.reshape(n_sup, 16, P, P).transpose(0, 2, 1, 3)
        ).reshape(n_sup, P, 2048)

        hk = hp[k * NPC:(k + 1) * NPC].reshape(NT, P, H)
        hTk = np.ascontiguousarray(
            hk.transpose(0, 2, 1).reshape(NT, 2, P, P).transpose(0, 2, 1, 3)
        ).reshape(NT, P, 2 * P).astype(BF16_NP)

        # velg cols per node tile: [comp j, gate k] at 5*j+k
        vg = (velp[k * NPC:(k + 1) * NPC]
              .reshape(NT, P, 5, 3).transpose(1, 0, 3, 2)
              .reshape(P, NT * 15))
        velg = np.ascontiguousarray(vg)

        in_maps.append({
            "mijT": mijT,
            "ohT": ohT,
            "rel": rel,
            "hT": hTk,
            "velg": velg,
            "w1dr": w1dr,
            "w2c": w2c,
            "b1t": b1t,
            "vgw1b": vgw1b,
            "vgw2t": vgw2t,
            "vgb1t": vgb1t,
            "onesb2": onesb2,
            "onesp": onesp,
        })
    return in_maps, Ks, b2, perms, invc


def unpack_out(arr: np.ndarray) -> np.ndarray:
    """[128, NT*3] packed per-core output -> [NPC, 3]."""
    return arr.reshape(P, NT, 3).transpose(1, 0, 2).reshape(NPC, 3)


def finalize(results, perms, invc) -> np.ndarray:
    """Combine per-core vel output + slot-ordered geom sums on the host."""
    out = np.empty((N_PAD, 3), np.float32)
    for k in range(N_CORES):
        vel = unpack_out(np.asarray(results[k]["out"], np.float32))
        geo = np.asarray(results[k]["geo"], np.float32)  # [3, NT*128]
        geoT = geo.T.reshape(NT, P, 3)
        geom = np.empty((NPC, 3), np.float32)
        perm = perms[k]
        for j in range(NT):
            geom[perm[j] * P:(perm[j] + 1) * P] = geoT[j]
        nodes = slice(k * NPC, (k + 1) * NPC)
        iv = np.ones(NPC, np.float32)
        n_real = min(N_NODES - k * NPC, NPC)
        if n_real > 0:
            iv[:n_real] = invc[k * NPC:k * NPC + n_real]
        out[nodes] = vel + geom * iv[:, None]
    return out[:N_NODES]


def kernel(**inputs) -> np.ndarray:
    in_maps, Ks, b2, perms, invc = _preprocess(inputs)
    nc = _build_program(Ks, b2)
    res = run_bass_kernel_spmd(nc, in_maps, list(range(N_CORES)))
    return finalize(res.results, perms, invc).astype(np.float32)
